# revision 1
# baseline (speedup 1.0000x reference)
"""Sharded k-NN retrieval kernel for Trainium2 (8 NeuronCores).

Problem: for each of 64 obs rows, find the 16 nearest memories (L2 over the
first 64 dims, obs L2-normalized), then return the action slice of the
candidate with the largest return-sum.

Strategy (row-sharded k-NN):
  - memories [1M, 88] sharded row-wise across 8 cores (125k rows each).
  - Host packs each shard as [65, 2L]: rows 0:64 = mem_obs^T, row 64 = ||m||^2
    (fp32), split into two column streams (A/B) so the PE can col-tile.
  - Device (per core, raw bass pipeline): scores = 2*obs_n . m - ||m||^2 via
    one K=65 fp32 matmul per 512-column chunk (two concurrent col-group
    streams), windowed max-pool (window 32) on DVE from PSUM, then per-row
    top-16 pooled windows (max8/match_replace/max_index).
  - Host: merges 8 cores' candidate windows, takes top-32 windows per obs
    row, exactly re-scores those rows (float64), takes the true top-16,
    then computes the ret-sum argmax and gathers the action.

A window containing any true top-16 row always has pooled-max >= the 16th
best score, and globally at most 16 such windows exist, so each one ranks
in its core-half's top-16 and survives the host's top-32 merge: the final
top-16 is exact (up to fp32 matmul noise on ~1e-4-separated ties).
"""
from contextlib import ExitStack

import numpy as np

import concourse.bass as bass
from concourse import mybir
from concourse.bass_utils import run_bass_kernel_spmd

F32 = mybir.dt.float32
BF16 = mybir.dt.bfloat16
U32 = mybir.dt.uint32

# problem constants (hardcoded for nn_BaseThinker_38766374814195)
N_MEMS = 1_000_000
MEM_DIM = 88
B = 64          # obs batch
D = 64          # obs dims used for distance
ACT_LEN = 16
RET_LEN = 8
K = 16
N_CORES = 8

COLTILE = 2048                    # columns per matmul tile
WIN = 32                          # pool window
L = 63488                         # columns per stream half = 31 * 2048
KDIM = D + 2                      # contraction: 64 bf16 dims + r_hi + r_lo
PAD_SENTINEL = 1.0e9              # r_hi for pad columns -> score ~ -1e9
HOST_TOPW = 32                    # windows kept per obs row after merge
NBUF_T = 3                        # stream tile buffers per stream
R_SHARD = N_MEMS // N_CORES       # 125000 rows per core


def _build_module(l_half: int = L):
    """Raw-bass pipeline; standalone wait_ge instructions (no Tile) keep
    every matmul/DMA under walrus's per-instruction sync-wait limit."""
    assert l_half % COLTILE == 0
    ntiles = l_half // COLTILE
    npool = l_half // WIN
    nwin = COLTILE // WIN

    nc = bass.Bass()
    w_dram = nc.dram_tensor("w", [KDIM, B], BF16, kind="ExternalInput")
    packed = nc.dram_tensor("packed", [KDIM, 2 * l_half], BF16,
                            kind="ExternalInput")
    vals_dram = nc.dram_tensor("vals16", [128, 16], F32, kind="ExternalOutput")
    idx_dram = nc.dram_tensor("idx16", [128, 16], U32, kind="ExternalOutput")

    with ExitStack() as ctx:
        w_sb = ctx.enter_context(nc.sbuf_tensor("w_sb", [KDIM, B], BF16))
        ta = [ctx.enter_context(nc.sbuf_tensor(f"ta{i}", [KDIM, COLTILE], BF16))
              for i in range(NBUF_T)]
        tb = [ctx.enter_context(nc.sbuf_tensor(f"tb{i}", [KDIM, COLTILE], BF16))
              for i in range(NBUF_T)]
        pooled = ctx.enter_context(nc.sbuf_tensor("pooled", [128, npool], F32))
        pooled2 = ctx.enter_context(nc.sbuf_tensor("pooled2", [128, npool], F32))
        v16 = ctx.enter_context(nc.sbuf_tensor("v16", [128, 16], F32))
        i16 = ctx.enter_context(nc.sbuf_tensor("i16", [128, 16], U32))
        ps = [ctx.enter_context(nc.psum_tensor(f"ps{i}", [128, COLTILE], F32))
              for i in range(2)]
        s_w = ctx.enter_context(nc.semaphore("s_w"))
        # one completion semaphore per stream buffer slot: a DMA's +16 is
        # 16 per-engine increments that interleave across in-flight
        # transfers, so a shared counter can't order completions
        s_da = [ctx.enter_context(nc.semaphore(f"s_da{i}"))
                for i in range(NBUF_T)]
        s_db = [ctx.enter_context(nc.semaphore(f"s_db{i}"))
                for i in range(NBUF_T)]
        s_pe = ctx.enter_context(nc.semaphore("s_pe"))
        s_dve = ctx.enter_context(nc.semaphore("s_dve"))
        s_out = ctx.enter_context(nc.semaphore("s_out"))
        blk = ctx.enter_context(nc.Block())

        @blk.sync
        def _(sync):
            # weights + stream A loads on the SP HWDGE queue
            sync.dma_start(w_sb[:], w_dram[:]).then_inc(s_w, 16)
            for t in range(ntiles):
                if t >= NBUF_T:
                    sync.wait_ge(s_pe, t - NBUF_T + 1)
                c0 = t * COLTILE
                sync.dma_start(ta[t % NBUF_T][:],
                               packed[:, c0:c0 + COLTILE]
                               ).then_inc(s_da[t % NBUF_T], 16)
            # results out
            sync.wait_ge(s_out, 1)
            sync.dma_start(vals_dram[:], v16[:]).then_inc(s_w, 16)
            sync.dma_start(idx_dram[:], i16[:]).then_inc(s_w, 16)

        @blk.scalar
        def _(scalar):
            # stream B loads on the ACT HWDGE queue
            for t in range(ntiles):
                if t >= NBUF_T:
                    scalar.wait_ge(s_pe, t - NBUF_T + 1)
                c0 = l_half + t * COLTILE
                scalar.dma_start(tb[t % NBUF_T][:],
                                 packed[:, c0:c0 + COLTILE]
                                 ).then_inc(s_db[t % NBUF_T], 16)

        @blk.tensor
        def _(pe):
            pe.wait_ge(s_w, 16)
            for t in range(ntiles):
                pe.wait_ge(s_da[t % NBUF_T], 16 * (t // NBUF_T + 1))
                pe.wait_ge(s_db[t % NBUF_T], 16 * (t // NBUF_T + 1))
                if t >= 2:
                    pe.wait_ge(s_dve, t - 1)
                pst = ps[t % 2]
                a_t, b_t = ta[t % NBUF_T], tb[t % NBUF_T]
                last = None
                for s in range(COLTILE // 512):
                    sl = slice(s * 512, (s + 1) * 512)
                    pe.matmul(pst[0:B, sl], w_sb[:], a_t[:, sl],
                              start=True, stop=True, tile_position=(0, 0))
                    last = pe.matmul(pst[B:128, sl], w_sb[:], b_t[:, sl],
                                     start=True, stop=True,
                                     tile_position=(0, 64))
                last.then_inc(s_pe, 1)

        @blk.vector
        def _(dve):
            for t in range(ntiles):
                dve.wait_ge(s_pe, t + 1)
                dve.tensor_reduce(
                    pooled[:, t * nwin:(t + 1) * nwin],
                    ps[t % 2][:].rearrange("p (n w) -> p n w", w=WIN),
                    axis=mybir.AxisListType.X, op=mybir.AluOpType.max,
                    opt_input=False,
                ).then_inc(s_dve, 1)
            # level 2: top-16 pooled windows per partition row. DVE ops
            # pipeline, so each dependent op needs a completion wait on
            # its producer (self-semaphore).
            dve.wait_ge(s_dve, ntiles)
            dve.max(v16[:, 0:8], pooled[:]).then_inc(s_dve, 1)
            dve.wait_ge(s_dve, ntiles + 1)
            dve.max_index(i16[:, 0:8], v16[:, 0:8],
                          pooled[:]).then_inc(s_dve, 1)
            dve.wait_ge(s_dve, ntiles + 2)
            dve.match_replace(pooled2[:], v16[:, 0:8], pooled[:],
                              -3.0e38).then_inc(s_dve, 1)
            dve.wait_ge(s_dve, ntiles + 3)
            dve.max(v16[:, 8:16], pooled2[:]).then_inc(s_dve, 1)
            dve.wait_ge(s_dve, ntiles + 4)
            dve.max_index(i16[:, 8:16], v16[:, 8:16],
                          pooled2[:]).then_inc(s_out, 1)

    return nc


# ---------------- host side ----------------

def _pack_shards(memories: np.ndarray) -> list[np.ndarray]:
    import ml_dtypes
    bf = ml_dtypes.bfloat16
    mem_obs_t = np.ascontiguousarray(memories[:, :D].T)          # [64, 1M]
    norms2 = np.einsum("dn,dn->n", mem_obs_t, mem_obs_t,
                       dtype=np.float32).astype(np.float32)       # [1M]
    # r = ||m||^2 - 64 split into bf16 hi+lo keeps the norm term accurate
    # to ~5e-4 while streaming in bf16; the -64 global shift cancels in
    # ranking. Device scores are thus (true score + 64) +- ~0.03, plenty
    # for window *selection* (host re-scores exactly).
    r = norms2 - np.float32(64.0)
    r_hi32 = r.astype(bf).astype(np.float32)
    r_lo = (r - r_hi32).astype(bf)
    mem_bf = mem_obs_t.astype(bf)
    shards = []
    for c in range(N_CORES):
        lo, hi = c * R_SHARD, (c + 1) * R_SHARD
        packed = np.zeros((KDIM, 2 * L), dtype=bf)
        packed[0:D, :R_SHARD] = mem_bf[:, lo:hi]
        packed[D, :R_SHARD] = r_hi32[lo:hi].astype(bf)
        packed[D, R_SHARD:] = bf(PAD_SENTINEL)
        packed[D + 1, :R_SHARD] = r_lo[lo:hi]
        shards.append(packed)
    return shards


def _finalize(memories: np.ndarray, obs: np.ndarray,
              vals: np.ndarray, idxs: np.ndarray) -> np.ndarray:
    """vals/idxs: [n_cores, 128, 16] device outputs -> best_acts [B, ACT_LEN]."""
    obs_n = obs.astype(np.float64)
    obs_n /= np.clip(np.linalg.norm(obs_n, axis=1, keepdims=True), 1e-12, None)

    # candidate windows per obs row: value + (core, local start col)
    # partition p: batch p%64, half p//64
    cand_vals = np.empty((B, N_CORES * 2 * 16), dtype=np.float32)
    cand_local = np.empty((B, N_CORES * 2 * 16), dtype=np.int64)
    cand_core = np.empty(N_CORES * 2 * 16, dtype=np.int64)
    for c in range(N_CORES):
        for half in range(2):
            p_sl = slice(half * 64, half * 64 + 64)
            v = vals[c][p_sl, :]                       # [64, 16]
            w = idxs[c][p_sl, :].astype(np.int64)      # [64, 16] window idx
            col = (c * 2 + half) * 16
            cand_vals[:, col:col + 16] = v
            cand_local[:, col:col + 16] = half * L + w * WIN
            cand_core[col:col + 16] = c

    top = np.argsort(-cand_vals, axis=1, kind="stable")[:, :HOST_TOPW]
    starts = np.take_along_axis(cand_local, top, axis=1)  # [B, HOST_TOPW]
    cores = cand_core[top]                                # [B, HOST_TOPW]

    mem64 = memories[:, :D]
    best_acts = np.empty((B, ACT_LEN), dtype=np.float32)
    offs = np.arange(WIN, dtype=np.int64)
    for b in range(B):
        local = (starts[b][:, None] + offs[None, :]).ravel()
        core = np.repeat(cores[b], WIN)
        valid = local < R_SHARD        # drop shard pad rows
        rows = np.unique(core[valid] * R_SHARD + local[valid])
        cm = mem64[rows].astype(np.float64)
        d2 = ((cm * cm).sum(axis=1) - 2.0 * (cm @ obs_n[b])
              + (obs_n[b] * obs_n[b]).sum())
        order = np.argsort(d2, kind="stable")[:K]
        top_rows = rows[order]
        ret_sum = memories[top_rows, D + ACT_LEN:].astype(np.float64).sum(axis=1)
        best = int(np.argmax(ret_sum))
        best_acts[b] = memories[top_rows[best], D:D + ACT_LEN]
    return best_acts


_CACHED_NC = None


def run_knn(inputs: dict, trace: bool = False):
    global _CACHED_NC
    obs = np.asarray(inputs["obs"], dtype=np.float32)
    memories = np.asarray(inputs["memories"], dtype=np.float32)
    assert obs.shape == (B, D) and memories.shape == (N_MEMS, MEM_DIM)
    assert int(inputs["obs_len"]) == D and int(inputs["act_len"]) == ACT_LEN
    assert int(inputs["k"]) == K

    shards = _pack_shards(memories)
    # weights: rows 0:64 = (2*obs_n)^T, row 64 = -1  (matches reference's
    # f.normalize: obs / clip(norm, eps))
    import ml_dtypes
    norm = np.clip(np.linalg.norm(obs, axis=1, keepdims=True), 1e-12, None)
    obs_n = (obs / norm).astype(np.float32)
    w = np.empty((KDIM, B), dtype=ml_dtypes.bfloat16)
    w[0:D, :] = (2.0 * obs_n).T.astype(ml_dtypes.bfloat16)
    w[D, :] = -1.0
    w[D + 1, :] = -1.0
    in_maps = [{"w": w, "packed": shards[c]} for c in range(N_CORES)]

    if _CACHED_NC is None:
        _CACHED_NC = _build_module()
    res = run_bass_kernel_spmd(_CACHED_NC, in_maps,
                               core_ids=list(range(N_CORES)), trace=trace)
    vals = np.stack([np.asarray(r["vals16"]) for r in res.results])
    idxs = np.stack([np.asarray(r["idx16"]) for r in res.results])
    out = _finalize(memories, obs, vals, idxs)
    return out, res.exec_time_ns


def kernel(**inputs) -> np.ndarray:
    out, _ = run_knn(inputs, trace=False)
    return out

